# revision 10
# baseline (speedup 1.0000x reference)
"""DecorrelatedBN (ZCA whitening) Trainium2 Bass kernel — 8-core data-parallel.

Problem: x [64,32,32,512] f32, NHWC, channel groups of m=64 (G=8 groups).
  out = ((x - mean) @ P) * gamma + beta,  P = (sigma + eps*I)^(-1/2) per group.

Sharding: rows (M = 65536) split contiguously across 8 cores (8192 each).
Local raw second moments + row sums are AllReduced ([128,516] f32), every
core computes P via Newton-Schulz (6 coupled iters; the real per-group
sigma has eigenvalues in [0.06, 2.03] so 6 iters reach ~9e-5) and applies
the projection locally.

v5 layout: 1024-row macro-tiles staged as [128, 4096] f32 with partition p
holding 8 consecutive DRAM rows -> all input/output DMA descriptors are
16KB contiguous. Row subsets j in 0..8 are independent 128-row tiles for
the PE (partition sums split arbitrarily).

Per-core dataflow:
  Phase A (8 macro-tiles): DMA in; DVE cast f32->bf16 into resident xb
    (pair-pitch 130 with a ones column at offset 128 -> sigma matmul with
    moving free=129 accumulates sigma AND the per-channel row sums in one
    PSUM tile); 4 pair matmuls per 128-row tile; some tiles' PE transposes
    (bf16, via identity) run inline, the rest fill the AllReduce window.
  AllReduce [128,516] f32 (4 pair blocks of [128, 128+1]).
  NS: A_p = mask.*(sig/M - mu mu^T) + eps I; 3 coupled NS iters (fp32);
    fold gamma into P, cast P->bf16; bias = beta - mu@P replicated.
  Phase B: white = xT_blk.T @ P_blk per tile (bf16 matmuls, fp32 PSUM),
    DVE adds bias during PSUM->SBUF evac, 16KB-line DMA out.
"""
import os
import sys

sys.path.insert(0, "/opt/trn_rl_repo")

import numpy as np
import concourse.bass as bass
import concourse.bacc as bacc
import concourse.tile as tile
import concourse.mybir as mybir
from concourse import bass_utils

dt = mybir.dt
Alu = mybir.AluOpType

# Problem constants (hardcoded per harness contract)
N, H, W, C = 64, 32, 32, 512
M_TOTAL = N * H * W          # 65536 rows
N_CORES = 8
M_LOC = M_TOTAL // N_CORES   # 8192 rows per core
EPS = 1e-5
GROUP = 64                   # channels per whitening group
N_PAIRS = 4                  # 8 groups packed as 4 pairs of [128,128] blocks

J_PER_MACRO = 8              # row-subsets per macro-tile (8 rows/partition)
N_MACROS = M_LOC // (128 * J_PER_MACRO)   # 8 macro-tiles of 1024 rows
N_TILES = N_MACROS * J_PER_MACRO          # 64 tile-equivalents
PITCH = 130                  # bf16 cols per pair block: 128 data + 1 ones + pad
XB_TILE = N_PAIRS * PITCH    # 520 bf16 cols per 128-row tile
NS_ITERS = 6
TR_FULL_MACROS = 6           # macros fully transposed inline in phase A;
                             # the rest transpose inside the AllReduce window

_CACHED = {}


def _build_bass():
    nc = bacc.Bacc("TRN2", target_bir_lowering=False, debug=False,
                   num_devices=N_CORES)
    f32 = dt.float32
    bf16 = dt.bfloat16

    x = nc.dram_tensor("x", [M_LOC, C], f32, kind="ExternalInput").ap()
    gamma = nc.dram_tensor("gamma", [1, C], f32, kind="ExternalInput").ap()
    beta = nc.dram_tensor("beta", [1, C], f32, kind="ExternalInput").ap()
    ident = nc.dram_tensor("ident", [128, 128], f32, kind="ExternalInput").ap()
    identb = nc.dram_tensor("identb", [128, 128], bf16, kind="ExternalInput").ap()
    mask_bd = nc.dram_tensor("mask_bd", [128, 128], f32, kind="ExternalInput").ap()
    eye15 = nc.dram_tensor("eye15", [128, 128], f32, kind="ExternalInput").ap()
    eps_eye = nc.dram_tensor("eps_eye", [128, 128], f32, kind="ExternalInput").ap()
    ones_row = nc.dram_tensor("ones_row", [1, 128], f32, kind="ExternalInput").ap()
    out = nc.dram_tensor("out", [M_LOC, C], f32, kind="ExternalOutput").ap()

    ROWS_PER_MACRO = 128 * J_PER_MACRO   # 1024

    with tile.TileContext(nc) as tc:
        with (
            tc.tile_pool(name="const", bufs=1) as constp,
            tc.tile_pool(name="resid", bufs=1) as residp,
            tc.tile_pool(name="small", bufs=1) as smallp,
            tc.tile_pool(name="dram", bufs=1, space="DRAM") as dramp,
        ):
            # ---- constants to SBUF ----
            id_sb = constp.tile([128, 128], f32, name="id_sb")
            idb_sb = constp.tile([128, 128], bf16, name="idb_sb")
            mask_sb = constp.tile([128, 128], f32, name="mask_sb")
            eye15_sb = constp.tile([128, 128], f32, name="eye15_sb")
            epseye_sb = constp.tile([128, 128], f32, name="epseye_sb")
            onesr_sb = constp.tile([1, 128], f32, name="onesr_sb")
            gamma_sb = constp.tile([1, C], f32, name="gamma_sb")
            beta_sb = constp.tile([1, C], f32, name="beta_sb")
            def load_consts():
                # issued after the first x chunk so the input stream starts
                # at t=0; nothing here is needed before ~10us into phase A
                nc.sync.dma_start(id_sb[:], ident[:])
                nc.sync.dma_start(idb_sb[:], identb[:])
                nc.sync.dma_start(mask_sb[:], mask_bd[:])
                nc.sync.dma_start(eye15_sb[:], eye15[:])
                nc.sync.dma_start(epseye_sb[:], eps_eye[:])
                nc.sync.dma_start(onesr_sb[:], ones_row[:])
                nc.sync.dma_start(gamma_sb[:], gamma[:])
                nc.sync.dma_start(beta_sb[:], beta[:])

            # resident bf16 cast of x: per tile t, pair b: data at
            # [t*520 + b*130, +128), ones col at +128.
            xb = residp.tile([128, N_TILES * XB_TILE], bf16, name="xb")
            # resident transposed x (bf16): tile t block b at [512t+128b, +128)
            xT = residp.tile([128, N_TILES * C], bf16, name="xT")

            # ones columns of xb (cols 128,129 of each 130-pitch block)
            xb_blocks = xb[:].rearrange("p (t e) -> p t e", e=PITCH)
            nc.vector.memset(xb_blocks[:, :, 128:130], 1.0)

            # PE warmup: HAM clock-gate releases only after ~3.4us of
            # sustained matmul activity; run throwaway bf16 matmuls so
            # phase A starts at 2.4 GHz.
            warm_sb = constp.tile([128, 512], bf16, name="warm_sb")
            nc.vector.memset(warm_sb[:], 0.5)
            with tc.tile_pool(name="warmps", bufs=1, space="PSUM") as warmpp:
                warm_ps = warmpp.tile([128, 512], f32, name="warm_ps")
                for _ in range(24):
                    nc.tensor.matmul(warm_ps[:], warm_sb[:, 0:128], warm_sb[:],
                                     start=True, stop=True)

            # ================= Phase A: stats + some transposes ===========
            def transpose_tile(t, trpp):
                """PE-transpose tile t's 4 blocks (bf16) and evac to xT."""
                tr = trpp.tile([128, C], bf16, tag="trps")
                for b in range(N_PAIRS):
                    nc.tensor.transpose(
                        tr[:, b * 128:(b + 1) * 128],
                        xb[:, t * XB_TILE + b * PITCH:
                           t * XB_TILE + b * PITCH + 128],
                        idb_sb[:])
                nc.scalar.copy(xT[:, t * C:(t + 1) * C], tr[:])

            # statsum holds the unpacked AllReduced stats; the off-diagonal
            # quadrants are never written (masked later) -> zero them once
            statsum = smallp.tile([128, N_PAIRS * 129], f32, name="statsum")
            nc.vector.memset(statsum[:], 0.0)

            with (
                tc.tile_pool(name="instage", bufs=3) as inp,
                tc.tile_pool(name="sigps", bufs=1, space="PSUM") as sigpp,
                tc.tile_pool(name="trpsA", bufs=3, space="PSUM") as trppA,
            ):
                sig_ps = [sigpp.tile([128, 129], f32, name=f"sig{p}",
                                     tag=f"sig{p}") for p in range(N_PAIRS)]

                for mt in range(N_MACROS):
                    stage = inp.tile([128, ROWS_PER_MACRO // 128 * C], f32,
                                     tag="instage")
                    src = x[mt * ROWS_PER_MACRO:(mt + 1) * ROWS_PER_MACRO, :]
                    # partition p <- 8 consecutive rows: 16KB contiguous lines
                    nc.sync.dma_start(
                        stage[:],
                        src.rearrange("(p j) c -> p (j c)", j=J_PER_MACRO))
                    if mt == 0:
                        load_consts()
                    # cast f32 -> bf16 into the 130-pitch resident layout
                    stage_v = stage[:].rearrange(
                        "p (j b e) -> p (j b) e", j=J_PER_MACRO, e=128)
                    xb_mt = xb[:, mt * J_PER_MACRO * XB_TILE:
                               (mt + 1) * J_PER_MACRO * XB_TILE]
                    xb_v = xb_mt.rearrange(
                        "p (t e) -> p t e", e=PITCH)[:, :, 0:128]
                    nc.vector.tensor_copy(xb_v, stage_v)

                    for j in range(J_PER_MACRO):
                        t = mt * J_PER_MACRO + j
                        first = (t == 0)
                        last = (t == N_TILES - 1)
                        for b in range(N_PAIRS):
                            off = t * XB_TILE + b * PITCH
                            nc.tensor.matmul(
                                sig_ps[b][:],
                                xb[:, off:off + 128],        # stationary
                                xb[:, off:off + 129],        # moving (+ones)
                                start=first, stop=last)
                        if mt < TR_FULL_MACROS:
                            transpose_tile(t, trppA)

                # pack stats for the wire: only the two 64x64 diagonal
                # quadrants + mean col per pair -> [128, 4*65] (133KB).
                # All copies are partition-aligned (even groups live on
                # partitions 0:64, odd on 64:128), so DVE can do them.
                packed = smallp.tile([128, N_PAIRS * 65], f32, name="packed")
                for p in range(N_PAIRS):
                    po = p * 65
                    nc.vector.tensor_copy(packed[0:64, po:po + 64],
                                          sig_ps[p][0:64, 0:64])
                    nc.vector.tensor_copy(packed[64:128, po:po + 64],
                                          sig_ps[p][64:128, 64:128])
                    nc.vector.tensor_copy(packed[0:64, po + 64:po + 65],
                                          sig_ps[p][0:64, 128:129])
                    nc.vector.tensor_copy(packed[64:128, po + 64:po + 65],
                                          sig_ps[p][64:128, 128:129])

            # ================= AllReduce =================
            ar_in = dramp.tile([128, N_PAIRS * 65], f32, name="ar_in")
            ar_out = dramp.tile([128, N_PAIRS * 65], f32, name="ar_out")
            nc.sync.dma_start(ar_in[:], packed[:])
            nc.gpsimd.collective_compute(
                "AllReduce", Alu.add,
                replica_groups=[list(range(N_CORES))],
                ins=[ar_in.opt()], outs=[ar_out.opt()],
            )
            packsum = smallp.tile([128, N_PAIRS * 65], f32, name="packsum")
            nc.sync.dma_start(packsum[:], ar_out[:])

            # Remaining transposes fill the AllReduce wait (and keep the
            # HAM clock warm).
            with tc.tile_pool(name="trpsB", bufs=3, space="PSUM") as trppB:
                for mt in range(TR_FULL_MACROS, N_MACROS):
                    for j in range(J_PER_MACRO):
                        transpose_tile(mt * J_PER_MACRO + j, trppB)

            # unpack AllReduced stats into statsum's diagonal quadrants
            for p in range(N_PAIRS):
                po, so = p * 65, p * 129
                nc.vector.tensor_copy(statsum[0:64, so:so + 64],
                                      packsum[0:64, po:po + 64])
                nc.vector.tensor_copy(statsum[64:128, so + 64:so + 128],
                                      packsum[64:128, po:po + 64])
                nc.vector.tensor_copy(statsum[0:64, so + 128:so + 129],
                                      packsum[0:64, po + 64:po + 65])
                nc.vector.tensor_copy(statsum[64:128, so + 128:so + 129],
                                      packsum[64:128, po + 64:po + 65])

            # a few dep-free matmuls to keep PE busy through the AR tail
            with tc.tile_pool(name="warmps2", bufs=1, space="PSUM") as warmpp2:
                warm2_ps = warmpp2.tile([128, 512], f32, name="warm2_ps")
                for _ in range(10):
                    nc.tensor.matmul(warm2_ps[:], warm_sb[:, 0:128],
                                     warm_sb[:], start=True, stop=True)

            # ================= small-matrix phase =================
            with tc.tile_pool(name="nsps", bufs=2, space="PSUM") as nspp:
                # mu columns [128, 4]: pair b's channel means (from the
                # ones-column of the sigma matmuls), scaled by 1/M
                mu_cols = smallp.tile([128, N_PAIRS], f32, name="mu_cols")
                statsum_v = statsum[:].rearrange("p (b e) -> p b e", e=129)
                nc.vector.tensor_scalar_mul(
                    mu_cols[:].rearrange("p (b e) -> p b e", e=1),
                    statsum_v[:, :, 128:129], 1.0 / M_TOTAL)
                # mu rows: one [1,128] tile per pair via PE transpose
                # (matmul operands need base partition 0/32/64)
                murow_sb = [smallp.tile([1, 128], f32, name=f"murow{p}")
                            for p in range(N_PAIRS)]
                for p in range(N_PAIRS):
                    murow_ps = nspp.tile([1, 128], f32, tag="ns0")
                    nc.tensor.transpose(murow_ps[:], mu_cols[:, p:p + 1],
                                        id_sb[:])
                    nc.vector.tensor_copy(murow_sb[p][:], murow_ps[:])

                P_sb = smallp.tile([128, C], f32, name="P_sb")
                Pb_sb = smallp.tile([128, C], bf16, name="Pb_sb")
                Y_sb = [smallp.tile([128, 128], f32, name=f"Y{p}")
                        for p in range(N_PAIRS)]
                Z_sb = [smallp.tile([128, 128], f32, name=f"Z{p}")
                        for p in range(N_PAIRS)]
                B_sb = [smallp.tile([128, 128], f32, name=f"B{p}")
                        for p in range(N_PAIRS)]

                # A_p = mask .* (sig_p/M - mu mu^T) + eps I ; Y=A, Z=I
                for p in range(N_PAIRS):
                    outer_ps = nspp.tile([128, 128], f32, tag="ns0")
                    nc.tensor.matmul(outer_ps[:], murow_sb[p][:],
                                     murow_sb[p][:], start=True, stop=True)
                    A = Y_sb[p]
                    nc.vector.scalar_tensor_tensor(
                        A[:], statsum[:, p * 129:p * 129 + 128], 1.0 / M_TOTAL,
                        outer_ps[:], op0=Alu.mult, op1=Alu.subtract)
                    nc.vector.tensor_tensor(A[:], A[:], mask_sb[:], op=Alu.mult)
                    nc.vector.tensor_tensor(A[:], A[:], epseye_sb[:], op=Alu.add)
                    nc.vector.tensor_copy(Z_sb[p][:], id_sb[:])

                # coupled Newton-Schulz: W=Z@Y; B=1.5I-0.5W; Y=Y@B; Z=B@Z
                for it in range(NS_ITERS):
                    for p in range(N_PAIRS):
                        w_ps = nspp.tile([128, 128], f32, tag="ns0")
                        nc.tensor.matmul(w_ps[:], Z_sb[p][:], Y_sb[p][:],
                                         start=True, stop=True)
                        nc.vector.scalar_tensor_tensor(
                            B_sb[p][:], w_ps[:], -0.5, eye15_sb[:],
                            op0=Alu.mult, op1=Alu.add)
                    for p in range(N_PAIRS):
                        z_ps = nspp.tile([128, 128], f32, tag="ns2")
                        nc.tensor.matmul(z_ps[:], B_sb[p][:], Z_sb[p][:],
                                         start=True, stop=True)
                        if it < NS_ITERS - 1:
                            y_ps = nspp.tile([128, 128], f32, tag="ns1")
                            nc.tensor.matmul(y_ps[:], Y_sb[p][:], B_sb[p][:],
                                             start=True, stop=True)
                            nc.scalar.copy(Y_sb[p][:], y_ps[:])
                        nc.vector.tensor_copy(Z_sb[p][:], z_ps[:])

                # gamma-fold: P = Z .* gamma_rep (column scale); cast bf16
                grep_ps = nspp.tile([128, C], f32, tag="grep")
                nc.tensor.matmul(grep_ps[:], onesr_sb[:], gamma_sb[:],
                                 start=True, stop=True)
                for p in range(N_PAIRS):
                    nc.vector.tensor_tensor(
                        P_sb[:, p * 128:(p + 1) * 128], Z_sb[p][:],
                        grep_ps[:, p * 128:(p + 1) * 128], op=Alu.mult)
                nc.vector.tensor_copy(Pb_sb[:], P_sb[:])

                # bias = beta - mu @ P (per pair), then replicate to 128 rows
                bias_row = smallp.tile([1, C], f32, name="bias_row")
                for p in range(N_PAIRS):
                    mp_ps = nspp.tile([1, 128], f32, tag="ns1")
                    nc.tensor.matmul(mp_ps[:], mu_cols[:, p:p + 1],
                                     P_sb[:, p * 128:(p + 1) * 128],
                                     start=True, stop=True)
                    nc.vector.scalar_tensor_tensor(
                        bias_row[0:1, p * 128:(p + 1) * 128], mp_ps[:], -1.0,
                        beta_sb[0:1, p * 128:(p + 1) * 128],
                        op0=Alu.mult, op1=Alu.add)
                bias_rep = smallp.tile([128, C], f32, name="bias_rep")
                brep_ps = nspp.tile([128, C], f32, tag="grep")
                nc.tensor.matmul(brep_ps[:], onesr_sb[:], bias_row[:],
                                 start=True, stop=True)
                nc.scalar.copy(bias_rep[:], brep_ps[:])

            # ================= Phase B: apply =================
            with (
                tc.tile_pool(name="outstage", bufs=2) as outp,
                tc.tile_pool(name="whps", bufs=3, space="PSUM") as whpp,
            ):
                for mt in range(N_MACROS):
                    ostage = outp.tile([128, J_PER_MACRO * C], f32,
                                       tag="outstage")
                    for j in range(J_PER_MACRO):
                        t = mt * J_PER_MACRO + j
                        wh = whpp.tile([128, C], f32, tag="whps")
                        if j % 2 == 1:
                            # bias pre-load into PSUM (K=1 outer product);
                            # evac is then a plain copy the scalar engine
                            # can do -> halves the DVE load in phase B
                            nc.tensor.matmul(wh[:], onesr_sb[:], bias_row[:],
                                             start=True, stop=False,
                                             skip_group_check=True)
                        for b in range(N_PAIRS):
                            nc.tensor.matmul(
                                wh[:, b * 128:(b + 1) * 128],
                                xT[:, t * C + b * 128: t * C + (b + 1) * 128],
                                Pb_sb[:, b * 128:(b + 1) * 128],
                                start=(j % 2 == 0), stop=True,
                                skip_group_check=True)
                        if j % 2 == 1:
                            nc.scalar.copy(ostage[:, j * C:(j + 1) * C], wh[:])
                        else:
                            nc.vector.tensor_tensor(
                                ostage[:, j * C:(j + 1) * C], wh[:],
                                bias_rep[:], op=Alu.add)
                        if j == J_PER_MACRO // 2 - 1 or j == J_PER_MACRO - 1:
                            # half-macro output DMAs: finer pipelining and a
                            # shorter drain on the last macro
                            h = 0 if j < J_PER_MACRO // 2 else 1
                            half = J_PER_MACRO // 2
                            dst = out[mt * ROWS_PER_MACRO:
                                      (mt + 1) * ROWS_PER_MACRO, :]
                            dst_v = dst.rearrange("(p j) c -> p j c",
                                                  j=J_PER_MACRO)
                            nc.sync.dma_start(
                                dst_v[:, h * half:(h + 1) * half, :],
                                ostage[:].rearrange(
                                    "p (j c) -> p j c", j=J_PER_MACRO)
                                [:, h * half:(h + 1) * half, :])

    nc.compile()
    return nc


def _get_nc():
    if "nc" not in _CACHED:
        _CACHED["nc"] = _build_bass()
    return _CACHED["nc"]


def _const_inputs():
    if "consts" not in _CACHED:
        ident = np.eye(128, dtype=np.float32)
        mask = np.zeros((128, 128), dtype=np.float32)
        mask[:GROUP, :GROUP] = 1.0
        mask[GROUP:, GROUP:] = 1.0
        _CACHED["consts"] = {
            "ident": ident,
            "identb": ident.astype(dt.np(dt.bfloat16)),
            "mask_bd": mask,
            "eye15": (1.5 * ident).astype(np.float32),
            "eps_eye": (EPS * ident).astype(np.float32),
            "ones_row": np.ones((1, 128), dtype=np.float32),
        }
    return _CACHED["consts"]


def kernel(x, gamma, beta, _trace=False):
    x = np.asarray(x, dtype=np.float32)
    gamma2 = np.ascontiguousarray(np.asarray(gamma, np.float32).reshape(1, C))
    beta2 = np.ascontiguousarray(np.asarray(beta, np.float32).reshape(1, C))
    xf = np.ascontiguousarray(x.reshape(M_TOTAL, C))

    consts = _const_inputs()
    in_maps = []
    for k in range(N_CORES):
        m = {"x": np.ascontiguousarray(xf[k * M_LOC:(k + 1) * M_LOC]),
             "gamma": gamma2, "beta": beta2}
        m.update(consts)
        in_maps.append(m)

    nc = _get_nc()
    res = bass_utils.run_bass_kernel_spmd(
        nc, in_maps, core_ids=list(range(N_CORES)), trace=_trace)
    out = np.concatenate([res.results[k]["out"] for k in range(N_CORES)], axis=0)
    out = out.reshape(N, H, W, C)
    if _trace:
        _CACHED["last_results"] = res
    return out


# revision 21
# speedup vs baseline: 1.2061x; 1.2061x over previous
"""DecorrelatedBN (ZCA whitening) Trainium2 Bass kernel — 8-core data-parallel.

Problem: x [64,32,32,512] f32, NHWC, channel groups of m=64 (G=8 groups).
  out = ((x - mean) @ P) * gamma + beta,  P = (sigma + eps*I)^(-1/2) per group.

Sharding: rows (M = 65536) split contiguously across 8 cores (8192 each).
Local raw second moments + row sums are AllReduced ([128,516] f32), every
core computes P via Newton-Schulz (6 coupled iters; the real per-group
sigma has eigenvalues in [0.06, 2.03] so 6 iters reach ~9e-5) and applies
the projection locally.

v5 layout: 1024-row macro-tiles staged as [128, 4096] f32 with partition p
holding 8 consecutive DRAM rows -> all input/output DMA descriptors are
16KB contiguous. Row subsets j in 0..8 are independent 128-row tiles for
the PE (partition sums split arbitrarily).

Per-core dataflow:
  Phase A (8 macro-tiles): DMA in; DVE cast f32->bf16 into resident xb
    (pair-pitch 130 with a ones column at offset 128 -> sigma matmul with
    moving free=129 accumulates sigma AND the per-channel row sums in one
    PSUM tile); 4 pair matmuls per 128-row tile; some tiles' PE transposes
    (bf16, via identity) run inline, the rest fill the AllReduce window.
  AllReduce [128,516] f32 (4 pair blocks of [128, 128+1]).
  NS: A_p = mask.*(sig/M - mu mu^T) + eps I; 3 coupled NS iters (fp32);
    fold gamma into P, cast P->bf16; bias = beta - mu@P replicated.
  Phase B: white = xT_blk.T @ P_blk per tile (bf16 matmuls, fp32 PSUM),
    DVE adds bias during PSUM->SBUF evac, 16KB-line DMA out.
"""
import os
import sys

sys.path.insert(0, "/opt/trn_rl_repo")

import numpy as np
import concourse.bass as bass
import concourse.bacc as bacc
import concourse.tile as tile
import concourse.mybir as mybir
from concourse import bass_utils

dt = mybir.dt
Alu = mybir.AluOpType

# Problem constants (hardcoded per harness contract)
N, H, W, C = 64, 32, 32, 512
M_TOTAL = N * H * W          # 65536 rows
N_CORES = 8
M_LOC = M_TOTAL // N_CORES   # 8192 rows per core
EPS = 1e-5
GROUP = 64                   # channels per whitening group
N_PAIRS = 4                  # 8 groups packed as 4 pairs of [128,128] blocks

J_PER_MACRO = 8              # row-subsets per macro-tile (8 rows/partition)
N_MACROS = M_LOC // (128 * J_PER_MACRO)   # 8 macro-tiles of 1024 rows
N_TILES = N_MACROS * J_PER_MACRO          # 64 tile-equivalents
PITCH = 130                  # bf16 cols per pair block: 128 data + 1 ones + pad
XB_TILE = N_PAIRS * PITCH    # 520 bf16 cols per 128-row tile
NS_ITERS = 6
# inline transposes per macro in phase A: enough to keep the PE ~85% busy
# (clock-gate avoidance) but a light tail so sigma reaches the AllReduce
# quickly; the complement runs inside the AllReduce window
TRJ_PER_MACRO = [4, 4, 4, 4, 4, 4, 2, 0]

_CACHED = {}


def _build_bass():
    nc = bacc.Bacc("TRN2", target_bir_lowering=False, debug=False,
                   num_devices=N_CORES)
    f32 = dt.float32
    bf16 = dt.bfloat16

    x = nc.dram_tensor("x", [M_LOC, C], f32, kind="ExternalInput").ap()
    gamma = nc.dram_tensor("gamma", [1, C], f32, kind="ExternalInput").ap()
    beta = nc.dram_tensor("beta", [1, C], f32, kind="ExternalInput").ap()
    ident = nc.dram_tensor("ident", [128, 128], f32, kind="ExternalInput").ap()
    identb = nc.dram_tensor("identb", [128, 128], bf16, kind="ExternalInput").ap()
    mask_bd = nc.dram_tensor("mask_bd", [128, 128], f32, kind="ExternalInput").ap()
    eye15 = nc.dram_tensor("eye15", [128, 128], f32, kind="ExternalInput").ap()
    eps_eye = nc.dram_tensor("eps_eye", [128, 128], f32, kind="ExternalInput").ap()
    ones_row = nc.dram_tensor("ones_row", [1, 128], f32, kind="ExternalInput").ap()
    out = nc.dram_tensor("out", [M_LOC, C], f32, kind="ExternalOutput").ap()

    ROWS_PER_MACRO = 128 * J_PER_MACRO   # 1024

    with tile.TileContext(nc) as tc:
        with (
            tc.tile_pool(name="const", bufs=1) as constp,
            tc.tile_pool(name="resid", bufs=1) as residp,
            tc.tile_pool(name="small", bufs=1) as smallp,
            tc.tile_pool(name="dram", bufs=1, space="DRAM") as dramp,
        ):
            # ---- constants to SBUF ----
            id_sb = constp.tile([128, 128], f32, name="id_sb")
            idb_sb = constp.tile([128, 128], bf16, name="idb_sb")
            mask_sb = constp.tile([128, 128], f32, name="mask_sb")
            eye15_sb = constp.tile([128, 128], f32, name="eye15_sb")
            epseye_sb = constp.tile([128, 128], f32, name="epseye_sb")
            onesr_sb = constp.tile([1, 128], f32, name="onesr_sb")
            gamma_sb = constp.tile([1, C], f32, name="gamma_sb")
            beta_sb = constp.tile([1, C], f32, name="beta_sb")
            def load_consts():
                # issued after the first x chunk so the input stream starts
                # at t=0; nothing here is needed before ~10us into phase A
                nc.sync.dma_start(id_sb[:], ident[:])
                nc.sync.dma_start(idb_sb[:], identb[:])
                nc.sync.dma_start(mask_sb[:], mask_bd[:])
                nc.sync.dma_start(eye15_sb[:], eye15[:])
                nc.sync.dma_start(epseye_sb[:], eps_eye[:])
                nc.sync.dma_start(onesr_sb[:], ones_row[:])
                nc.sync.dma_start(gamma_sb[:], gamma[:])
                nc.sync.dma_start(beta_sb[:], beta[:])

            # resident bf16 cast of x: per tile t, pair b: data at
            # [t*520 + b*130, +128), ones col at +128.
            xb = residp.tile([128, N_TILES * XB_TILE], bf16, name="xb")
            # resident transposed x (bf16): tile t block b at [512t+128b, +128)
            xT = residp.tile([128, N_TILES * C], bf16, name="xT")

            # ones columns of xb (cols 128,129 of each 130-pitch block)
            xb_blocks = xb[:].rearrange("p (t e) -> p t e", e=PITCH)
            nc.vector.memset(xb_blocks[:, :, 128:130], 1.0)

            # PE warmup: HAM clock-gate releases only after ~3.4us of
            # sustained matmul activity; run throwaway bf16 matmuls so
            # phase A starts at 2.4 GHz.
            warm_sb = constp.tile([128, 512], bf16, name="warm_sb")
            nc.vector.memset(warm_sb[:], 0.5)
            def warm(pool, n, free=512, tag="warm"):
                """Dep-free filler matmuls: keep the PE stream continuous so
                the pstate/HAM clock stays at 2.4 GHz through gaps."""
                for _ in range(n):
                    wps = pool.tile([128, free], f32, tag=tag)
                    nc.tensor.matmul(wps[:], warm_sb[:, 0:128],
                                     warm_sb[:, 0:free], start=True, stop=True)

            with tc.tile_pool(name="warmps", bufs=1, space="PSUM") as warmpp:
                warm(warmpp, 24)

            # ================= Phase A: stats + some transposes ===========
            def transpose_tile(t, trpp):
                """PE-transpose tile t's 4 blocks (bf16) and evac to xT."""
                tr = trpp.tile([128, C], bf16, tag="trps")
                for b in range(N_PAIRS):
                    nc.tensor.transpose(
                        tr[:, b * 128:(b + 1) * 128],
                        xb[:, t * XB_TILE + b * PITCH:
                           t * XB_TILE + b * PITCH + 128],
                        idb_sb[:])
                nc.scalar.copy(xT[:, t * C:(t + 1) * C], tr[:])

            # statsum holds the unpacked AllReduced stats; the off-diagonal
            # quadrants are never written (masked later) -> zero them once
            statsum = smallp.tile([128, N_PAIRS * 129], f32, name="statsum")
            nc.vector.memset(statsum[:], 0.0)

            with (
                tc.tile_pool(name="instage", bufs=3) as inp,
                tc.tile_pool(name="sigps", bufs=1, space="PSUM") as sigpp,
                tc.tile_pool(name="trpsA", bufs=3, space="PSUM") as trppA,
            ):
                sig_ps = [sigpp.tile([128, 129], f32, name=f"sig{p}",
                                     tag=f"sig{p}") for p in range(N_PAIRS)]

                for mt in range(N_MACROS):
                    stage = inp.tile([128, ROWS_PER_MACRO // 128 * C], f32,
                                     tag="instage")
                    src = x[mt * ROWS_PER_MACRO:(mt + 1) * ROWS_PER_MACRO, :]
                    # partition p <- 8 consecutive rows: 16KB contiguous lines
                    nc.sync.dma_start(
                        stage[:],
                        src.rearrange("(p j) c -> p (j c)", j=J_PER_MACRO))
                    if mt == 0:
                        load_consts()
                    # cast f32 -> bf16 into the 130-pitch resident layout
                    stage_v = stage[:].rearrange(
                        "p (j b e) -> p (j b) e", j=J_PER_MACRO, e=128)
                    xb_mt = xb[:, mt * J_PER_MACRO * XB_TILE:
                               (mt + 1) * J_PER_MACRO * XB_TILE]
                    xb_v = xb_mt.rearrange(
                        "p (t e) -> p t e", e=PITCH)[:, :, 0:128]
                    nc.vector.tensor_copy(xb_v, stage_v)

                    for j in range(J_PER_MACRO):
                        t = mt * J_PER_MACRO + j
                        first = (t == 0)
                        last = (t == N_TILES - 1)
                        for b in range(N_PAIRS):
                            off = t * XB_TILE + b * PITCH
                            nc.tensor.matmul(
                                sig_ps[b][:],
                                xb[:, off:off + 128],        # stationary
                                xb[:, off:off + 129],        # moving (+ones)
                                start=first, stop=last)
                        if j < TRJ_PER_MACRO[mt]:
                            transpose_tile(t, trppA)

                # pack stats for the wire: only the two 64x64 diagonal
                # quadrants + mean col per pair -> [128, 4*65] (133KB).
                # All copies are partition-aligned (even groups live on
                # partitions 0:64, odd on 64:128), so DVE can do them.
                packed = smallp.tile([128, N_PAIRS * 65], f32, name="packed")
                for p in range(N_PAIRS):
                    po = p * 65
                    nc.vector.tensor_copy(packed[0:64, po:po + 64],
                                          sig_ps[p][0:64, 0:64])
                    nc.vector.tensor_copy(packed[64:128, po:po + 64],
                                          sig_ps[p][64:128, 64:128])
                    nc.vector.tensor_copy(packed[0:64, po + 64:po + 65],
                                          sig_ps[p][0:64, 128:129])
                    nc.vector.tensor_copy(packed[64:128, po + 64:po + 65],
                                          sig_ps[p][64:128, 128:129])

            # ================= AllReduce =================
            ar_in = dramp.tile([128, N_PAIRS * 65], f32, name="ar_in")
            ar_out = dramp.tile([128, N_PAIRS * 65], f32, name="ar_out")
            nc.sync.dma_start(ar_in[:], packed[:])
            nc.gpsimd.collective_compute(
                "AllReduce", Alu.add,
                replica_groups=[list(range(N_CORES))],
                ins=[ar_in.opt()], outs=[ar_out.opt()],
            )
            packsum = smallp.tile([128, N_PAIRS * 65], f32, name="packsum")
            nc.sync.dma_start(packsum[:], ar_out[:])

            # Remaining transposes fill the AllReduce wait (and keep the
            # HAM clock warm); filler matmuls cover the rest of the window.
            with tc.tile_pool(name="trpsB", bufs=3, space="PSUM") as trppB:
                for mt in range(N_MACROS):
                    for j in range(TRJ_PER_MACRO[mt], J_PER_MACRO):
                        transpose_tile(mt * J_PER_MACRO + j, trppB)
                warm(trppB, 50)

            # unpack AllReduced stats into statsum's diagonal quadrants
            for p in range(N_PAIRS):
                po, so = p * 65, p * 129
                nc.vector.tensor_copy(statsum[0:64, so:so + 64],
                                      packsum[0:64, po:po + 64])
                nc.vector.tensor_copy(statsum[64:128, so + 64:so + 128],
                                      packsum[64:128, po:po + 64])
                nc.vector.tensor_copy(statsum[0:64, so + 128:so + 129],
                                      packsum[0:64, po + 64:po + 65])
                nc.vector.tensor_copy(statsum[64:128, so + 128:so + 129],
                                      packsum[64:128, po + 64:po + 65])

            # ================= small-matrix phase =================
            with tc.tile_pool(name="nsps", bufs=2, space="PSUM") as nspp:
                # mu columns [128, 4]: pair b's channel means (from the
                # ones-column of the sigma matmuls), scaled by 1/M
                mu_cols = smallp.tile([128, N_PAIRS], f32, name="mu_cols")
                statsum_v = statsum[:].rearrange("p (b e) -> p b e", e=129)
                nc.vector.tensor_scalar_mul(
                    mu_cols[:].rearrange("p (b e) -> p b e", e=1),
                    statsum_v[:, :, 128:129], 1.0 / M_TOTAL)
                # mu rows: one [1,128] tile per pair via PE transpose
                # (matmul operands need base partition 0/32/64)
                murow_sb = [smallp.tile([1, 128], f32, name=f"murow{p}")
                            for p in range(N_PAIRS)]
                for p in range(N_PAIRS):
                    murow_ps = nspp.tile([1, 128], f32, tag="ns0")
                    nc.tensor.transpose(murow_ps[:], mu_cols[:, p:p + 1],
                                        id_sb[:])
                    nc.vector.tensor_copy(murow_sb[p][:], murow_ps[:])

                P_sb = smallp.tile([128, C], f32, name="P_sb")
                Pb_sb = smallp.tile([128, C], bf16, name="Pb_sb")
                Y_sb = [smallp.tile([128, 128], f32, name=f"Y{p}")
                        for p in range(N_PAIRS)]
                Z_sb = [smallp.tile([128, 128], f32, name=f"Z{p}")
                        for p in range(N_PAIRS)]
                B_sb = [smallp.tile([128, 128], f32, name=f"B{p}")
                        for p in range(N_PAIRS)]

                # A_p = mask .* (sig_p/M - mu mu^T) + eps I ; Y=A, Z=I
                for p in range(N_PAIRS):
                    outer_ps = nspp.tile([128, 128], f32, tag="ns0")
                    nc.tensor.matmul(outer_ps[:], murow_sb[p][:],
                                     murow_sb[p][:], start=True, stop=True)
                    A = Y_sb[p]
                    nc.vector.scalar_tensor_tensor(
                        A[:], statsum[:, p * 129:p * 129 + 128], 1.0 / M_TOTAL,
                        outer_ps[:], op0=Alu.mult, op1=Alu.subtract)
                    nc.vector.tensor_tensor(A[:], A[:], mask_sb[:], op=Alu.mult)
                    nc.vector.tensor_tensor(A[:], A[:], epseye_sb[:], op=Alu.add)
                    nc.vector.tensor_copy(Z_sb[p][:], id_sb[:])

                # coupled Newton-Schulz: W=Z@Y; B=1.5I-0.5W; Y=Y@B; Z=B@Z
                for it in range(NS_ITERS):
                    for p in range(N_PAIRS):
                        w_ps = nspp.tile([128, 128], f32, tag="ns0")
                        nc.tensor.matmul(w_ps[:], Z_sb[p][:], Y_sb[p][:],
                                         start=True, stop=True)
                        nc.vector.scalar_tensor_tensor(
                            B_sb[p][:], w_ps[:], -0.5, eye15_sb[:],
                            op0=Alu.mult, op1=Alu.add)
                    warm(nspp, 3, tag="grep")  # bridge PE gap during DVE B
                    for p in range(N_PAIRS):
                        z_ps = nspp.tile([128, 128], f32, tag="ns2")
                        nc.tensor.matmul(z_ps[:], B_sb[p][:], Z_sb[p][:],
                                         start=True, stop=True)
                        if it < NS_ITERS - 1:
                            y_ps = nspp.tile([128, 128], f32, tag="ns1")
                            nc.tensor.matmul(y_ps[:], Y_sb[p][:], B_sb[p][:],
                                             start=True, stop=True)
                            nc.scalar.copy(Y_sb[p][:], y_ps[:])
                        nc.vector.tensor_copy(Z_sb[p][:], z_ps[:])

                # gamma-fold: P = Z .* gamma_rep (column scale); cast bf16
                grep_ps = nspp.tile([128, C], f32, tag="grep")
                nc.tensor.matmul(grep_ps[:], onesr_sb[:], gamma_sb[:],
                                 start=True, stop=True)
                for p in range(N_PAIRS):
                    nc.vector.tensor_tensor(
                        P_sb[:, p * 128:(p + 1) * 128], Z_sb[p][:],
                        grep_ps[:, p * 128:(p + 1) * 128], op=Alu.mult)
                nc.vector.tensor_copy(Pb_sb[:], P_sb[:])

                # bias = beta - mu @ P (per pair), then replicate to 128 rows
                bias_row = smallp.tile([1, C], f32, name="bias_row")
                for p in range(N_PAIRS):
                    mp_ps = nspp.tile([1, 128], f32, tag="ns1")
                    nc.tensor.matmul(mp_ps[:], mu_cols[:, p:p + 1],
                                     P_sb[:, p * 128:(p + 1) * 128],
                                     start=True, stop=True)
                    nc.vector.scalar_tensor_tensor(
                        bias_row[0:1, p * 128:(p + 1) * 128], mp_ps[:], -1.0,
                        beta_sb[0:1, p * 128:(p + 1) * 128],
                        op0=Alu.mult, op1=Alu.add)
                bias_rep = smallp.tile([128, C], f32, name="bias_rep")
                brep_ps = nspp.tile([128, C], f32, tag="grep")
                nc.tensor.matmul(brep_ps[:], onesr_sb[:], bias_row[:],
                                 start=True, stop=True)
                nc.scalar.copy(bias_rep[:], brep_ps[:])

            # ================= Phase B: apply =================
            with (
                tc.tile_pool(name="outstage", bufs=2) as outp,
                tc.tile_pool(name="whps", bufs=3, space="PSUM") as whpp,
            ):
                for mt in range(N_MACROS):
                    ostage = outp.tile([128, J_PER_MACRO * C], f32,
                                       tag="outstage")
                    for j in range(J_PER_MACRO):
                        t = mt * J_PER_MACRO + j
                        wh = whpp.tile([128, C], f32, tag="whps")
                        for b in range(N_PAIRS):
                            nc.tensor.matmul(
                                wh[:, b * 128:(b + 1) * 128],
                                xT[:, t * C + b * 128: t * C + (b + 1) * 128],
                                Pb_sb[:, b * 128:(b + 1) * 128],
                                start=True, stop=True)
                        warm(whpp, 1, free=384)  # hold PE clock between tiles
                        nc.vector.tensor_tensor(
                            ostage[:, j * C:(j + 1) * C], wh[:],
                            bias_rep[:], op=Alu.add)
                        if j == J_PER_MACRO // 2 - 1 or j == J_PER_MACRO - 1:
                            # half-macro output DMAs: finer pipelining and a
                            # shorter drain on the last macro
                            h = 0 if j < J_PER_MACRO // 2 else 1
                            half = J_PER_MACRO // 2
                            dst = out[mt * ROWS_PER_MACRO:
                                      (mt + 1) * ROWS_PER_MACRO, :]
                            dst_v = dst.rearrange("(p j) c -> p j c",
                                                  j=J_PER_MACRO)
                            nc.sync.dma_start(
                                dst_v[:, h * half:(h + 1) * half, :],
                                ostage[:].rearrange(
                                    "p (j c) -> p j c", j=J_PER_MACRO)
                                [:, h * half:(h + 1) * half, :])

    nc.compile()
    return nc


def _get_nc():
    if "nc" not in _CACHED:
        _CACHED["nc"] = _build_bass()
    return _CACHED["nc"]


def _const_inputs():
    if "consts" not in _CACHED:
        ident = np.eye(128, dtype=np.float32)
        mask = np.zeros((128, 128), dtype=np.float32)
        mask[:GROUP, :GROUP] = 1.0
        mask[GROUP:, GROUP:] = 1.0
        _CACHED["consts"] = {
            "ident": ident,
            "identb": ident.astype(dt.np(dt.bfloat16)),
            "mask_bd": mask,
            "eye15": (1.5 * ident).astype(np.float32),
            "eps_eye": (EPS * ident).astype(np.float32),
            "ones_row": np.ones((1, 128), dtype=np.float32),
        }
    return _CACHED["consts"]


def kernel(x, gamma, beta, _trace=False):
    x = np.asarray(x, dtype=np.float32)
    gamma2 = np.ascontiguousarray(np.asarray(gamma, np.float32).reshape(1, C))
    beta2 = np.ascontiguousarray(np.asarray(beta, np.float32).reshape(1, C))
    xf = np.ascontiguousarray(x.reshape(M_TOTAL, C))

    consts = _const_inputs()
    in_maps = []
    for k in range(N_CORES):
        m = {"x": np.ascontiguousarray(xf[k * M_LOC:(k + 1) * M_LOC]),
             "gamma": gamma2, "beta": beta2}
        m.update(consts)
        in_maps.append(m)

    nc = _get_nc()
    res = bass_utils.run_bass_kernel_spmd(
        nc, in_maps, core_ids=list(range(N_CORES)), trace=_trace)
    out = np.concatenate([res.results[k]["out"] for k in range(N_CORES)], axis=0)
    out = out.reshape(N, H, W, C)
    if _trace:
        _CACHED["last_results"] = res
    return out
